# revision 4
# baseline (speedup 1.0000x reference)
"""MultiEdgeGraphBlock kernel for 8 Trainium2 NeuronCores — v2.

Sharding: nodes across cores (1250/core, padded 1280), ALL batches per core.
The gather table fuses all 4 batches per node into one 2048-byte bf16 row
(row m = [h[0,m,:] h[1,m,:] h[2,m,:] h[3,m,:]]), so each SWDGE gather
descriptor moves 2048B instead of 512B and the per-core descriptor count
drops 4x vs the (batch, node-half) sharding.

Per core, per 128-node block (10 blocks):
  - for each edge type: one 2048-index dma_gather -> G[128, 16, 1024] bf16
    (node-major; j = d*128 + n index layout).
  - DEG-sum on PE: 16 identity matmuls x 2 psum halves (free=512).
  - mean via ACT eviction with per-partition reciprocal-degree scale.
  - PE transpose to feature-major (8 x 128x128), then aggregated^T
    accumulates W_i^T @ meanT in PSUM over edges.
  - LayerNorm feature-major: stats via ones-vector matmuls; mu/rstd
    broadcast across partitions with a K=1 ones-row matmul (no DRAM trip).
  - MLP as feature-major bf16 matmuls; residual add in f32.
Columns within a block: col = b*128 + n  (512 LN/MLP columns per block).
"""

import sys

sys.path.insert(0, "/opt/trn_rl_repo")

import numpy as np
import ml_dtypes
from contextlib import ExitStack

import concourse.bass as bass
import concourse.mybir as mybir
import concourse.tile as tile
from concourse import bacc
from concourse.bass_utils import run_bass_kernel_spmd

BF16 = ml_dtypes.bfloat16
F32 = mybir.dt.float32
BF = mybir.dt.bfloat16
I16 = mybir.dt.int16
AO = mybir.AluOpType
AF = mybir.ActivationFunctionType

B, N, F, E, DEG, H = 4, 10000, 256, 5, 16, 256
NCORES = 8
NLOC = N // NCORES     # 1250 nodes per core
NPAD = 1280            # padded to 10 blocks of 128
NBLK = NPAD // 128     # 10
COLS = NPAD * B        # 5120 LN/MLP columns per core
BLK = 512              # columns per block (128 nodes x 4 batches)
ZROW = N               # zero-row index in gather table
EW = B * F             # 1024 elements per fused table row
GIDX = DEG * 128       # 2048 indices per gather call
NQUEUES = 4
LN_EPS = 1e-6

_PROGRAM = {}
VARIANT = "full"  # full | gather | nogather
FP8 = True        # gather table in fp8 e4m3 (halves gather HBM traffic)
TD = mybir.dt.float8e4
TDH = ml_dtypes.float8_e4m3


def _build_program(repeat=1):
    nc = bacc.Bacc(
        "TRN2",
        target_bir_lowering=False,
        debug=False,
        dynamic_dma_scratch_size=32768,
        num_swdge_queues=NQUEUES,
    )

    tbl = nc.dram_tensor("tbl", [N + 1, EW], TD if FP8 else BF, kind="ExternalInput")
    hT = nc.dram_tensor("hT", [F, COLS], F32, kind="ExternalInput")
    idxw = nc.dram_tensor("idxw", [E, NBLK, 128, 128], I16, kind="ExternalInput")
    masknm = nc.dram_tensor("masknm", [128, E, NBLK, DEG], F32, kind="ExternalInput")
    w_pe = nc.dram_tensor("w_pe", [128, E * 4, 128], BF, kind="ExternalInput")
    w1 = nc.dram_tensor("w1", [128, 8, 128], BF, kind="ExternalInput")
    w2 = nc.dram_tensor("w2", [128, 4, 128], BF, kind="ExternalInput")
    ident_d = nc.dram_tensor("ident", [128, 128], BF, kind="ExternalInput")
    identq_d = nc.dram_tensor("identq", [128, 128], TD, kind="ExternalInput")
    ones1_d = nc.dram_tensor("ones1", [128, 1], BF, kind="ExternalInput")
    onesr_d = nc.dram_tensor("onesr", [1, 128], F32, kind="ExternalInput")
    b1_d = nc.dram_tensor("b1pc", [128, 2], F32, kind="ExternalInput")
    b2_d = nc.dram_tensor("b2pc", [128, 2], F32, kind="ExternalInput")
    lns_d = nc.dram_tensor("lnspc", [128, 4], F32, kind="ExternalInput")
    lnb_d = nc.dram_tensor("lnbpc", [128, 4], F32, kind="ExternalInput")
    bedg_d = nc.dram_tensor("bedgpc", [128, 2, E], F32, kind="ExternalInput")

    outT = nc.dram_tensor("outT", [F, COLS], F32, kind="ExternalOutput")

    with tile.TileContext(nc) as tc, ExitStack() as ctx:
        cpool = ctx.enter_context(tc.tile_pool(name="const", bufs=1))
        spsum = ctx.enter_context(tc.tile_pool(name="spsum", bufs=2, space="PSUM"))
        mtpsum = ctx.enter_context(tc.tile_pool(name="mtpsum", bufs=2, space="PSUM"))
        apsum = ctx.enter_context(tc.tile_pool(name="apsum", bufs=1, space="PSUM"))
        mlpsum = ctx.enter_context(tc.tile_pool(name="mlpsum", bufs=1, space="PSUM"))
        gpool = ctx.enter_context(tc.tile_pool(name="g", bufs=3))
        ipool = ctx.enter_context(tc.tile_pool(name="idx", bufs=4))
        xpool = ctx.enter_context(tc.tile_pool(name="x", bufs=2))
        wpool = ctx.enter_context(tc.tile_pool(name="work", bufs=2))

        # ---------------- constants ----------------
        W_sb = cpool.tile([128, E * 4, 128], BF)
        nc.sync.dma_start(W_sb[:], w_pe[:])
        W1_sb = cpool.tile([128, 8, 128], BF)
        nc.sync.dma_start(W1_sb[:], w1[:])
        W2_sb = cpool.tile([128, 4, 128], BF)
        nc.sync.dma_start(W2_sb[:], w2[:])
        id_sb = cpool.tile([128, 128], BF)
        nc.sync.dma_start(id_sb[:], ident_d[:])
        idq_sb = cpool.tile([128, 128], TD)
        nc.sync.dma_start(idq_sb[:], identq_d[:])
        on_sb = cpool.tile([128, 1], BF)
        nc.sync.dma_start(on_sb[:], ones1_d[:])
        onr_sb = cpool.tile([1, 128], F32)
        nc.sync.dma_start(onr_sb[:], onesr_d[:])
        b1_sb = cpool.tile([128, 2], F32)
        nc.sync.dma_start(b1_sb[:], b1_d[:])
        b2_sb = cpool.tile([128, 2], F32)
        nc.sync.dma_start(b2_sb[:], b2_d[:])
        lns_sb = cpool.tile([128, 4], F32)
        nc.sync.dma_start(lns_sb[:], lns_d[:])
        lnb_sb = cpool.tile([128, 4], F32)
        nc.sync.dma_start(lnb_sb[:], lnb_d[:])
        bedg_sb = cpool.tile([128, 2, E], F32)
        nc.sync.dma_start(bedg_sb[:], bedg_d[:])
        bsum_sb = cpool.tile([128, 2], F32)
        nc.vector.tensor_reduce(
            bsum_sb[:], bedg_sb[:], axis=mybir.AxisListType.X, op=AO.add
        )

        # ---------------- reciprocal degree (node-major) ----------------
        mask_sb = cpool.tile([128, E, NBLK, DEG], F32)
        nc.sync.dma_start(mask_sb[:], masknm[:])
        dn_sb = cpool.tile([128, E, NBLK], F32)
        for i in range(E):
            nc.vector.tensor_reduce(
                dn_sb[:, i, :], mask_sb[:, i], axis=mybir.AxisListType.X, op=AO.add
            )
        nc.vector.tensor_scalar_max(dn_sb[:], dn_sb[:], 1.0)
        recip_sb = cpool.tile([128, E, NBLK], F32)
        nc.vector.reciprocal(recip_sb[:], dn_sb[:])

        qc = 0  # SWDGE queue round-robin counter

        # ---------------- main loop over node blocks ----------------
        for rep in range(repeat):
            for blk in range(NBLK):
                ns = bass.ts(blk, BLK)
                x = xpool.tile([128, 4, BLK], F32)
                nc.sync.dma_start(x[:, 0, :], hT[0:128, ns])
                nc.sync.dma_start(x[:, 1, :], hT[128:256, ns])

                agg = apsum.tile([128, 2, BLK], F32, tag="agg")
                for i in range(E):
                    idx_t = ipool.tile([128, 128], I16)
                    nc.sync.dma_start(idx_t[:], idxw[i, blk])
                    G = gpool.tile([128, DEG, EW], TD if FP8 else BF, tag="G")
                    ga = G[:]
                    gap = bass.AP(
                        ga.tensor, ga.offset, [ga.ap[0], [EW, DEG], [1, EW]]
                    )
                    if VARIANT == "nogather":
                        nc.vector.memset(G[:, 0, 0:8], 0.0)
                    else:
                        nc.gpsimd.dma_gather(
                            out_ap=gap,
                            in_ap=tbl.ap(),
                            idxs_ap=idx_t[:],
                            num_idxs=GIDX,
                            num_idxs_reg=GIDX,
                            elem_size=EW,
                            single_packet=False,
                            queue_num=qc % NQUEUES,
                        )
                    qc += 1
                    if VARIANT == "gather":
                        continue
                    mT = mtpsum.tile([128, 8, 128], BF, tag="mT")
                    for h2 in range(2):
                        S = spsum.tile([128, BLK], F32, tag="S")
                        for d in range(DEG):
                            nc.tensor.matmul(
                                S[:],
                                idq_sb[:] if FP8 else id_sb[:],
                                G[:, d, h2 * 512 : (h2 + 1) * 512],
                                start=(d == 0),
                                stop=(d == DEG - 1),
                            )
                        # mean (node-major): per-partition reciprocal scale
                        mean = wpool.tile([128, BLK], BF, tag="mean")
                        nc.scalar.activation(
                            mean[:], S[:], AF.Copy,
                            scale=recip_sb[:, i, blk : blk + 1],
                        )
                        # transpose reduced means to feature-major
                        for q in range(4):
                            nc.tensor.transpose(
                                mT[:, h2 * 4 + q, :],
                                mean[:, q * 128 : (q + 1) * 128],
                                id_sb[:],
                            )
                    mT_sb = wpool.tile([128, 8, 128], BF, tag="mTsb")
                    nc.scalar.copy(mT_sb[:], mT[:])
                    # aggregated^T += W_i^T @ meanT  (per batch, f-half)
                    for b4 in range(B):
                        for m in range(2):
                            for c in range(2):
                                nc.tensor.matmul(
                                    agg[:, m, b4 * 128 : (b4 + 1) * 128],
                                    W_sb[:, (i * 2 + c) * 2 + m, :],
                                    mT_sb[:, b4 * 2 + c, :],
                                    start=(i == 0 and b4 == 0 and c == 0),
                                    stop=(i == E - 1 and b4 == B - 1 and c == 1),
                                )
                if VARIANT == "gather":
                    continue
                # aggregated -> x bottom half (+ sum of edge biases)
                for m in range(2):
                    nc.scalar.activation(
                        x[:, 2 + m, :],
                        agg[:, m, :],
                        AF.Identity,
                        bias=bsum_sb[:, m : m + 1],
                        scale=1.0,
                    )

                # ---------------- layer norm (features on partitions) ------
                st = mlpsum.tile([128, 2, BLK], F32, tag="mlp")
                xbs = []
                for c in range(4):
                    xb = wpool.tile([128, BLK], BF, tag=f"xb{c}")
                    nc.vector.tensor_copy(xb[:], x[:, c, :])
                    xbs.append(xb)
                    nc.tensor.matmul(
                        st[0:1, 0, :], on_sb[:], xb[:],
                        start=(c == 0), stop=(c == 3),
                    )
                for c in range(4):
                    xsq = wpool.tile([128, BLK], BF, tag="xsq")
                    nc.scalar.square(xsq[:], xbs[c][:])
                    nc.tensor.matmul(
                        st[0:1, 1, :], on_sb[:], xsq[:],
                        start=(c == 0), stop=(c == 3),
                    )
                mrow = wpool.tile([1, 2, BLK], F32, tag="mrow")
                nc.vector.tensor_scalar_mul(mrow[0:1, 0, :], st[0:1, 0, :], 1.0 / 512.0)
                mu2 = wpool.tile([1, BLK], F32, tag="mu2")
                nc.vector.tensor_mul(mu2[0:1, :], mrow[0:1, 0, :], mrow[0:1, 0, :])
                nc.vector.tensor_scalar_sub(mu2[0:1, :], mu2[0:1, :], LN_EPS)
                var = wpool.tile([1, BLK], F32, tag="var")
                nc.vector.scalar_tensor_tensor(
                    var[0:1, :], st[0:1, 1, :], 1.0 / 512.0, mu2[0:1, :],
                    op0=AO.mult, op1=AO.subtract,
                )
                sd = wpool.tile([1, BLK], F32, tag="sd")
                nc.scalar.activation(sd[0:1, :], var[0:1, :], AF.Sqrt, bias=0.0)
                nc.vector.reciprocal(mrow[0:1, 1, :], sd[0:1, :])
                # broadcast mu/rstd to all 128 partitions via K=1 matmul
                bc = mlpsum.tile([128, 2, BLK], F32, tag="mlp")
                for r in range(2):
                    nc.tensor.matmul(
                        bc[:, r, :], onr_sb[:], mrow[0:1, r, :],
                        start=True, stop=True,
                    )

                xln = wpool.tile([128, 4, BLK], BF, tag="xln")
                for c in range(4):
                    tt = wpool.tile([128, BLK], F32, tag="tt")
                    nc.vector.scalar_tensor_tensor(
                        tt[:], x[:, c, :], 0.0, bc[:, 0, :],
                        op0=AO.add, op1=AO.subtract,
                    )
                    nc.vector.tensor_mul(tt[:], tt[:], bc[:, 1, :])
                    nc.scalar.activation(
                        xln[:, c, :], tt[:], AF.Identity,
                        bias=lnb_sb[:, c : c + 1], scale=lns_sb[:, c : c + 1],
                    )

                # ---------------- MLP ----------------
                y1 = mlpsum.tile([128, 2, BLK], F32, tag="mlp")
                for m in range(2):
                    for k in range(4):
                        nc.tensor.matmul(
                            y1[:, m, :], W1_sb[:, k * 2 + m, :], xln[:, k, :],
                            start=(k == 0), stop=(k == 3),
                        )
                y1b = wpool.tile([128, 2, BLK], BF, tag="y1b")
                for m in range(2):
                    nc.scalar.activation(
                        y1b[:, m, :], y1[:, m, :], AF.Relu,
                        bias=b1_sb[:, m : m + 1], scale=1.0,
                    )
                y2 = mlpsum.tile([128, 2, BLK], F32, tag="mlp")
                for m in range(2):
                    for k in range(2):
                        nc.tensor.matmul(
                            y2[:, m, :], W2_sb[:, k * 2 + m, :], y1b[:, k, :],
                            start=(k == 0), stop=(k == 1),
                        )
                ot = wpool.tile([128, 2, BLK], F32, tag="ot")
                for m in range(2):
                    nc.vector.scalar_tensor_tensor(
                        ot[:, m, :], y2[:, m, :], b2_sb[:, m : m + 1], x[:, m, :],
                        op0=AO.add, op1=AO.add,
                    )
                for m in range(2):
                    nc.sync.dma_start(outT[m * 128 : (m + 1) * 128, ns], ot[:, m, :])

    nc.compile()
    return nc


def _get_program(repeat=1):
    key = (repeat, VARIANT)
    if key not in _PROGRAM:
        _PROGRAM[key] = _build_program(repeat)
    return _PROGRAM[key]


def _prep_shared(edge_indices, edge_masks, W_edge, b_edge, ln_scale, ln_bias,
                 W1, b1, W2, b2):
    """Host-side layout prep: shared weights + per-core index/mask slices."""
    W_pe = np.empty((128, E * 4, 128), np.float32)
    for i in range(E):
        for c in range(2):
            for m in range(2):
                W_pe[:, (i * 2 + c) * 2 + m, :] = W_edge[
                    i, c * 128 : (c + 1) * 128, m * 128 : (m + 1) * 128
                ]
    W1b = np.empty((128, 8, 128), np.float32)
    for k in range(4):
        for m in range(2):
            W1b[:, k * 2 + m, :] = W1[k * 128 : (k + 1) * 128, m * 128 : (m + 1) * 128]
    W2b = np.empty((128, 4, 128), np.float32)
    for k in range(2):
        for m in range(2):
            W2b[:, k * 2 + m, :] = W2[k * 128 : (k + 1) * 128, m * 128 : (m + 1) * 128]

    shared = dict(
        w_pe=W_pe.astype(BF16),
        w1=W1b.astype(BF16),
        w2=W2b.astype(BF16),
        ident=np.eye(128, dtype=BF16),
        identq=np.eye(128, dtype=TDH),
        ones1=np.ones((128, 1), BF16),
        onesr=np.ones((1, 128), np.float32),
        b1pc=np.ascontiguousarray(b1.reshape(2, 128).T.astype(np.float32)),
        b2pc=np.ascontiguousarray(b2.reshape(2, 128).T.astype(np.float32)),
        lnspc=np.ascontiguousarray(ln_scale.reshape(4, 128).T.astype(np.float32)),
        lnbpc=np.ascontiguousarray(ln_bias.reshape(4, 128).T.astype(np.float32)),
        bedgpc=np.ascontiguousarray(
            b_edge.T.reshape(2, 128, E).transpose(1, 0, 2).astype(np.float32)
        ),
    )

    # per core: gather indices (mask-select -> zero row), node-major masks
    percore = []
    for core in range(NCORES):
        n0 = core * NLOC
        idx = edge_indices[:, n0 : n0 + NLOC, :].astype(np.int64)  # [E, NLOC, DEG]
        msk = edge_masks[:, n0 : n0 + NLOC, :]
        idx = np.where(idx < 0, 0, idx)
        sel = np.where(msk > 0, idx, ZROW).astype(np.int32)
        sel = np.concatenate(
            [sel, np.full((E, NPAD - NLOC, DEG), ZROW, np.int32)], axis=1
        )  # [E, NPAD, DEG]
        # per (edge type, 128-node block): j = d*128 + n, wrapped, replicated
        selT = sel.transpose(0, 2, 1)  # [E, DEG, NPAD]
        blocks = selT.reshape(E, DEG, NBLK, 128).transpose(0, 2, 1, 3)
        L = blocks.reshape(E, NBLK, GIDX)  # j = d*128 + n
        Wv = L.reshape(E, NBLK, GIDX // 16, 16).transpose(0, 1, 3, 2)
        idxw = np.tile(Wv, (1, 1, 8, 1)).astype(np.int16)  # [E, NBLK, 128, 128]

        mpad = np.concatenate(
            [msk, np.zeros((E, NPAD - NLOC, DEG), np.float32)], axis=1
        )  # [E, NPAD, DEG]
        masknm = np.ascontiguousarray(
            mpad.reshape(E, NBLK, 128, DEG).transpose(2, 0, 1, 3).astype(np.float32)
        )  # [128, E, NBLK, DEG]
        percore.append((idxw, masknm))
    return shared, percore


def _prep_core(h, tbl, shared, percore, core):
    n0 = core * NLOC
    hp = np.zeros((B, NPAD, F), np.float32)
    hp[:, :NLOC, :] = h[:, n0 : n0 + NLOC, :]
    # hT[f, col] with col = blk*512 + b*128 + n
    hTl = np.ascontiguousarray(
        hp.reshape(B, NBLK, 128, F).transpose(3, 1, 0, 2).reshape(F, COLS)
    )
    idxw, masknm = percore[core]
    m = dict(tbl=tbl, hT=hTl, idxw=idxw, masknm=masknm)
    m.update(shared)
    return m


def _make_table(h):
    dt = TDH if FP8 else BF16
    tbl = np.zeros((N + 1, EW), dt)
    tbl[:N] = h.transpose(1, 0, 2).reshape(N, EW).astype(dt)
    return tbl


def kernel(**inputs):
    h = np.asarray(inputs["h"], np.float32)
    nc = _get_program()
    shared, percore = _prep_shared(
        np.asarray(inputs["edge_indices"]),
        np.asarray(inputs["edge_masks"], np.float32),
        np.asarray(inputs["W_edge"], np.float32),
        np.asarray(inputs["b_edge"], np.float32),
        np.asarray(inputs["ln_scale"], np.float32),
        np.asarray(inputs["ln_bias"], np.float32),
        np.asarray(inputs["W1"], np.float32),
        np.asarray(inputs["b1"], np.float32),
        np.asarray(inputs["W2"], np.float32),
        np.asarray(inputs["b2"], np.float32),
    )
    tbl = _make_table(h)
    in_maps = [_prep_core(h, tbl, shared, percore, core) for core in range(NCORES)]

    res = run_bass_kernel_spmd(nc, in_maps, core_ids=list(range(NCORES)))

    out = np.empty((B, N, F), np.float32)
    for core in range(NCORES):
        n0 = core * NLOC
        o = res.results[core]["outT"]  # [256, COLS]
        arr = o.reshape(F, NBLK, B, 128).transpose(2, 1, 3, 0).reshape(B, NPAD, F)
        out[:, n0 : n0 + NLOC, :] = arr[:, :NLOC, :]
    return out


# revision 8
# speedup vs baseline: 1.2402x; 1.2402x over previous
"""MultiEdgeGraphBlock kernel for 8 Trainium2 NeuronCores — v2.

Sharding: nodes across cores (1250/core, padded 1280), ALL batches per core.
The gather table fuses all 4 batches per node into one 2048-byte bf16 row
(row m = [h[0,m,:] h[1,m,:] h[2,m,:] h[3,m,:]]), so each SWDGE gather
descriptor moves 2048B instead of 512B and the per-core descriptor count
drops 4x vs the (batch, node-half) sharding.

Per core, per 128-node block (10 blocks):
  - for each edge type: one 2048-index dma_gather -> G[128, 16, 1024] bf16
    (node-major; j = d*128 + n index layout).
  - DEG-sum on PE: 16 identity matmuls x 2 psum halves (free=512).
  - mean via ACT eviction with per-partition reciprocal-degree scale.
  - PE transpose to feature-major (8 x 128x128), then aggregated^T
    accumulates W_i^T @ meanT in PSUM over edges.
  - LayerNorm feature-major: stats via ones-vector matmuls; mu/rstd
    broadcast across partitions with a K=1 ones-row matmul (no DRAM trip).
  - MLP as feature-major bf16 matmuls; residual add in f32.
Columns within a block: col = b*128 + n  (512 LN/MLP columns per block).
"""

import os
import sys

sys.path.insert(0, "/opt/trn_rl_repo")

import numpy as np
import ml_dtypes
from contextlib import ExitStack

import concourse.bass as bass
import concourse.mybir as mybir
import concourse.tile as tile
from concourse import bacc
from concourse.bass_utils import run_bass_kernel_spmd

BF16 = ml_dtypes.bfloat16
F32 = mybir.dt.float32
BF = mybir.dt.bfloat16
I16 = mybir.dt.int16
AO = mybir.AluOpType
AF = mybir.ActivationFunctionType

B, N, F, E, DEG, H = 4, 10000, 256, 5, 16, 256
NCORES = 8
NLOC = N // NCORES     # 1250 nodes per core
NPAD = 1280            # padded to 10 blocks of 128
NBLK = NPAD // 128     # 10
COLS = NPAD * B        # 5120 LN/MLP columns per core
BLK = 512              # columns per block (128 nodes x 4 batches)
ZROW = N               # zero-row index in gather table
EW = B * F             # 1024 elements per fused table row
GIDX = DEG * 128       # 2048 indices per gather call
TPAD = 9               # compact: gather tiles of 128 rows per (edge, block)
CIDX = TPAD * 128      # 1152 compacted indices per gather call
NQUEUES = 4
LN_EPS = 1e-6

_PROGRAM = {}
VARIANT = "full"  # full | gather | nogather
FP8 = True        # gather table in fp8 e4m3 (halves gather HBM traffic)
SCRATCH = int(os.environ.get("KSCRATCH", "32768"))
SINGLE_PACKET = os.environ.get("KSP", "0") == "1"

TD = mybir.dt.float8e4
TDH = ml_dtypes.float8_e4m3


def _build_program(repeat=1):
    nc = bacc.Bacc(
        "TRN2",
        target_bir_lowering=False,
        debug=False,
        dynamic_dma_scratch_size=SCRATCH,
        num_swdge_queues=NQUEUES,
    )

    tbl = nc.dram_tensor("tbl", [N + 1, EW], TD if FP8 else BF, kind="ExternalInput")
    hT = nc.dram_tensor("hT", [F, COLS], F32, kind="ExternalInput")
    idxw = nc.dram_tensor("idxw", [E, NBLK, 128, CIDX // 16], I16, kind="ExternalInput")
    amat = nc.dram_tensor(
        "amat", [NBLK, 128, E, TPAD, 128], TD if FP8 else BF, kind="ExternalInput"
    )
    masknm = nc.dram_tensor("masknm", [128, E, NBLK, DEG], F32, kind="ExternalInput")
    w_pe = nc.dram_tensor("w_pe", [128, E * 4, 128], BF, kind="ExternalInput")
    w1 = nc.dram_tensor("w1", [128, 8, 128], BF, kind="ExternalInput")
    w2 = nc.dram_tensor("w2", [128, 4, 128], BF, kind="ExternalInput")
    ident_d = nc.dram_tensor("ident", [128, 128], BF, kind="ExternalInput")
    identq_d = nc.dram_tensor("identq", [128, 128], TD, kind="ExternalInput")
    ones1_d = nc.dram_tensor("ones1", [128, 1], BF, kind="ExternalInput")
    onesr_d = nc.dram_tensor("onesr", [1, 128], F32, kind="ExternalInput")
    b1_d = nc.dram_tensor("b1pc", [128, 2], F32, kind="ExternalInput")
    b2_d = nc.dram_tensor("b2pc", [128, 2], F32, kind="ExternalInput")
    lns_d = nc.dram_tensor("lnspc", [128, 4], F32, kind="ExternalInput")
    lnb_d = nc.dram_tensor("lnbpc", [128, 4], F32, kind="ExternalInput")
    bedg_d = nc.dram_tensor("bedgpc", [128, 2, E], F32, kind="ExternalInput")

    outT = nc.dram_tensor("outT", [F, COLS], F32, kind="ExternalOutput")

    with tile.TileContext(nc) as tc, ExitStack() as ctx:
        cpool = ctx.enter_context(tc.tile_pool(name="const", bufs=1))
        spsum = ctx.enter_context(tc.tile_pool(name="spsum", bufs=2, space="PSUM"))
        mtpsum = ctx.enter_context(tc.tile_pool(name="mtpsum", bufs=2, space="PSUM"))
        apsum = ctx.enter_context(tc.tile_pool(name="apsum", bufs=1, space="PSUM"))
        mlpsum = ctx.enter_context(tc.tile_pool(name="mlpsum", bufs=1, space="PSUM"))
        gpool = ctx.enter_context(tc.tile_pool(name="g", bufs=3))
        ipool = ctx.enter_context(tc.tile_pool(name="idx", bufs=4))
        xpool = ctx.enter_context(tc.tile_pool(name="x", bufs=2))
        apool = ctx.enter_context(tc.tile_pool(name="amat", bufs=2))
        wpool = ctx.enter_context(tc.tile_pool(name="work", bufs=2))

        # ---------------- constants ----------------
        W_sb = cpool.tile([128, E * 4, 128], BF)
        nc.sync.dma_start(W_sb[:], w_pe[:])
        W1_sb = cpool.tile([128, 8, 128], BF)
        nc.sync.dma_start(W1_sb[:], w1[:])
        W2_sb = cpool.tile([128, 4, 128], BF)
        nc.sync.dma_start(W2_sb[:], w2[:])
        id_sb = cpool.tile([128, 128], BF)
        nc.sync.dma_start(id_sb[:], ident_d[:])
        idq_sb = cpool.tile([128, 128], TD)
        nc.sync.dma_start(idq_sb[:], identq_d[:])
        on_sb = cpool.tile([128, 1], BF)
        nc.sync.dma_start(on_sb[:], ones1_d[:])
        onr_sb = cpool.tile([1, 128], F32)
        nc.sync.dma_start(onr_sb[:], onesr_d[:])
        b1_sb = cpool.tile([128, 2], F32)
        nc.sync.dma_start(b1_sb[:], b1_d[:])
        b2_sb = cpool.tile([128, 2], F32)
        nc.sync.dma_start(b2_sb[:], b2_d[:])
        lns_sb = cpool.tile([128, 4], F32)
        nc.sync.dma_start(lns_sb[:], lns_d[:])
        lnb_sb = cpool.tile([128, 4], F32)
        nc.sync.dma_start(lnb_sb[:], lnb_d[:])
        bedg_sb = cpool.tile([128, 2, E], F32)
        nc.sync.dma_start(bedg_sb[:], bedg_d[:])
        bsum_sb = cpool.tile([128, 2], F32)
        nc.vector.tensor_reduce(
            bsum_sb[:], bedg_sb[:], axis=mybir.AxisListType.X, op=AO.add
        )

        # ---------------- reciprocal degree (node-major) ----------------
        mask_sb = cpool.tile([128, E, NBLK, DEG], F32)
        nc.sync.dma_start(mask_sb[:], masknm[:])
        dn_sb = cpool.tile([128, E, NBLK], F32)
        for i in range(E):
            nc.vector.tensor_reduce(
                dn_sb[:, i, :], mask_sb[:, i], axis=mybir.AxisListType.X, op=AO.add
            )
        nc.vector.tensor_scalar_max(dn_sb[:], dn_sb[:], 1.0)
        recip_sb = cpool.tile([128, E, NBLK], F32)
        nc.vector.reciprocal(recip_sb[:], dn_sb[:])

        qc = 0  # SWDGE queue round-robin counter

        # ---------------- main loop over node blocks ----------------
        for rep in range(repeat):
            for blk in range(NBLK):
                ns = bass.ts(blk, BLK)
                x = xpool.tile([128, 4, BLK], F32)
                nc.sync.dma_start(x[:, 0, :], hT[0:128, ns])
                nc.sync.dma_start(x[:, 1, :], hT[128:256, ns])

                A_sb = apool.tile([128, E, TPAD, 128], TD if FP8 else BF, tag="A")
                nc.sync.dma_start(A_sb[:], amat[blk])
                agg = apsum.tile([128, 2, BLK], F32, tag="agg")
                for i in range(E):
                    idx_t = ipool.tile([128, CIDX // 16], I16)
                    nc.sync.dma_start(idx_t[:], idxw[i, blk])
                    G = gpool.tile([128, TPAD, EW], TD if FP8 else BF, tag="G")
                    ga = G[:]
                    gap = bass.AP(
                        ga.tensor, ga.offset, [ga.ap[0], [EW, TPAD], [1, EW]]
                    )
                    if VARIANT == "nogather":
                        nc.vector.memset(G[:, 0, 0:8], 0.0)
                    else:
                        nc.gpsimd.dma_gather(
                            out_ap=gap,
                            in_ap=tbl.ap(),
                            idxs_ap=idx_t[:],
                            num_idxs=CIDX,
                            num_idxs_reg=CIDX,
                            elem_size=EW,
                            single_packet=SINGLE_PACKET,
                            queue_num=qc % NQUEUES,
                        )
                    qc += 1
                    if VARIANT == "gather":
                        continue
                    mT = mtpsum.tile([128, 8, 128], BF, tag="mT")
                    for h2 in range(2):
                        S = spsum.tile([128, BLK], F32, tag="S")
                        for t in range(TPAD):
                            nc.tensor.matmul(
                                S[:],
                                A_sb[:, i, t, :],
                                G[:, t, h2 * 512 : (h2 + 1) * 512],
                                start=(t == 0),
                                stop=(t == TPAD - 1),
                            )
                        # mean (node-major): per-partition reciprocal scale
                        mean = wpool.tile([128, BLK], BF, tag="mean")
                        nc.scalar.activation(
                            mean[:], S[:], AF.Copy,
                            scale=recip_sb[:, i, blk : blk + 1],
                        )
                        # transpose reduced means to feature-major
                        for q in range(4):
                            nc.tensor.transpose(
                                mT[:, h2 * 4 + q, :],
                                mean[:, q * 128 : (q + 1) * 128],
                                id_sb[:],
                            )
                    mT_sb = wpool.tile([128, 8, 128], BF, tag="mTsb")
                    nc.scalar.copy(mT_sb[:], mT[:])
                    # aggregated^T += W_i^T @ meanT  (per batch, f-half)
                    for b4 in range(B):
                        for m in range(2):
                            for c in range(2):
                                nc.tensor.matmul(
                                    agg[:, m, b4 * 128 : (b4 + 1) * 128],
                                    W_sb[:, (i * 2 + c) * 2 + m, :],
                                    mT_sb[:, b4 * 2 + c, :],
                                    start=(i == 0 and b4 == 0 and c == 0),
                                    stop=(i == E - 1 and b4 == B - 1 and c == 1),
                                )
                if VARIANT == "gather":
                    continue
                # aggregated -> x bottom half (+ sum of edge biases)
                for m in range(2):
                    nc.scalar.activation(
                        x[:, 2 + m, :],
                        agg[:, m, :],
                        AF.Identity,
                        bias=bsum_sb[:, m : m + 1],
                        scale=1.0,
                    )

                # ---------------- layer norm (features on partitions) ------
                st = mlpsum.tile([128, 2, BLK], F32, tag="mlp")
                xbs = []
                for c in range(4):
                    xb = wpool.tile([128, BLK], BF, tag=f"xb{c}")
                    nc.vector.tensor_copy(xb[:], x[:, c, :])
                    xbs.append(xb)
                    nc.tensor.matmul(
                        st[0:1, 0, :], on_sb[:], xb[:],
                        start=(c == 0), stop=(c == 3),
                    )
                for c in range(4):
                    xsq = wpool.tile([128, BLK], BF, tag="xsq")
                    nc.scalar.square(xsq[:], xbs[c][:])
                    nc.tensor.matmul(
                        st[0:1, 1, :], on_sb[:], xsq[:],
                        start=(c == 0), stop=(c == 3),
                    )
                mrow = wpool.tile([1, 2, BLK], F32, tag="mrow")
                nc.vector.tensor_scalar_mul(mrow[0:1, 0, :], st[0:1, 0, :], 1.0 / 512.0)
                mu2 = wpool.tile([1, BLK], F32, tag="mu2")
                nc.vector.tensor_mul(mu2[0:1, :], mrow[0:1, 0, :], mrow[0:1, 0, :])
                nc.vector.tensor_scalar_sub(mu2[0:1, :], mu2[0:1, :], LN_EPS)
                var = wpool.tile([1, BLK], F32, tag="var")
                nc.vector.scalar_tensor_tensor(
                    var[0:1, :], st[0:1, 1, :], 1.0 / 512.0, mu2[0:1, :],
                    op0=AO.mult, op1=AO.subtract,
                )
                sd = wpool.tile([1, BLK], F32, tag="sd")
                nc.scalar.activation(sd[0:1, :], var[0:1, :], AF.Sqrt, bias=0.0)
                nc.vector.reciprocal(mrow[0:1, 1, :], sd[0:1, :])
                # broadcast mu/rstd to all 128 partitions via K=1 matmul
                bc = mlpsum.tile([128, 2, BLK], F32, tag="mlp")
                for r in range(2):
                    nc.tensor.matmul(
                        bc[:, r, :], onr_sb[:], mrow[0:1, r, :],
                        start=True, stop=True,
                    )

                xln = wpool.tile([128, 4, BLK], BF, tag="xln")
                for c in range(4):
                    tt = wpool.tile([128, BLK], F32, tag="tt")
                    nc.vector.scalar_tensor_tensor(
                        tt[:], x[:, c, :], 0.0, bc[:, 0, :],
                        op0=AO.add, op1=AO.subtract,
                    )
                    nc.vector.tensor_mul(tt[:], tt[:], bc[:, 1, :])
                    nc.scalar.activation(
                        xln[:, c, :], tt[:], AF.Identity,
                        bias=lnb_sb[:, c : c + 1], scale=lns_sb[:, c : c + 1],
                    )

                # ---------------- MLP ----------------
                y1 = mlpsum.tile([128, 2, BLK], F32, tag="mlp")
                for m in range(2):
                    for k in range(4):
                        nc.tensor.matmul(
                            y1[:, m, :], W1_sb[:, k * 2 + m, :], xln[:, k, :],
                            start=(k == 0), stop=(k == 3),
                        )
                y1b = wpool.tile([128, 2, BLK], BF, tag="y1b")
                for m in range(2):
                    nc.scalar.activation(
                        y1b[:, m, :], y1[:, m, :], AF.Relu,
                        bias=b1_sb[:, m : m + 1], scale=1.0,
                    )
                y2 = mlpsum.tile([128, 2, BLK], F32, tag="mlp")
                for m in range(2):
                    for k in range(2):
                        nc.tensor.matmul(
                            y2[:, m, :], W2_sb[:, k * 2 + m, :], y1b[:, k, :],
                            start=(k == 0), stop=(k == 1),
                        )
                ot = wpool.tile([128, 2, BLK], F32, tag="ot")
                for m in range(2):
                    nc.vector.scalar_tensor_tensor(
                        ot[:, m, :], y2[:, m, :], b2_sb[:, m : m + 1], x[:, m, :],
                        op0=AO.add, op1=AO.add,
                    )
                for m in range(2):
                    nc.sync.dma_start(outT[m * 128 : (m + 1) * 128, ns], ot[:, m, :])

    nc.compile()
    return nc


def _get_program(repeat=1):
    key = (repeat, VARIANT)
    if key not in _PROGRAM:
        _PROGRAM[key] = _build_program(repeat)
    return _PROGRAM[key]


def _prep_shared(edge_indices, edge_masks, W_edge, b_edge, ln_scale, ln_bias,
                 W1, b1, W2, b2):
    """Host-side layout prep: shared weights + per-core index/mask slices."""
    W_pe = np.empty((128, E * 4, 128), np.float32)
    for i in range(E):
        for c in range(2):
            for m in range(2):
                W_pe[:, (i * 2 + c) * 2 + m, :] = W_edge[
                    i, c * 128 : (c + 1) * 128, m * 128 : (m + 1) * 128
                ]
    W1b = np.empty((128, 8, 128), np.float32)
    for k in range(4):
        for m in range(2):
            W1b[:, k * 2 + m, :] = W1[k * 128 : (k + 1) * 128, m * 128 : (m + 1) * 128]
    W2b = np.empty((128, 4, 128), np.float32)
    for k in range(2):
        for m in range(2):
            W2b[:, k * 2 + m, :] = W2[k * 128 : (k + 1) * 128, m * 128 : (m + 1) * 128]

    shared = dict(
        w_pe=W_pe.astype(BF16),
        w1=W1b.astype(BF16),
        w2=W2b.astype(BF16),
        ident=np.eye(128, dtype=BF16),
        identq=np.eye(128, dtype=TDH),
        ones1=np.ones((128, 1), BF16),
        onesr=np.ones((1, 128), np.float32),
        b1pc=np.ascontiguousarray(b1.reshape(2, 128).T.astype(np.float32)),
        b2pc=np.ascontiguousarray(b2.reshape(2, 128).T.astype(np.float32)),
        lnspc=np.ascontiguousarray(ln_scale.reshape(4, 128).T.astype(np.float32)),
        lnbpc=np.ascontiguousarray(ln_bias.reshape(4, 128).T.astype(np.float32)),
        bedgpc=np.ascontiguousarray(
            b_edge.T.reshape(2, 128, E).transpose(1, 0, 2).astype(np.float32)
        ),
    )

    # per core: gather indices (mask-select -> zero row), node-major masks
    percore = []
    for core in range(NCORES):
        n0 = core * NLOC
        idx = edge_indices[:, n0 : n0 + NLOC, :].astype(np.int64)  # [E, NLOC, DEG]
        msk = edge_masks[:, n0 : n0 + NLOC, :]
        idx = np.where(idx < 0, 0, idx)
        sel = np.where(msk > 0, idx, ZROW).astype(np.int32)
        sel = np.concatenate(
            [sel, np.full((E, NPAD - NLOC, DEG), ZROW, np.int32)], axis=1
        )  # [E, NPAD, DEG]
        mpad = np.concatenate(
            [msk, np.zeros((E, NPAD - NLOC, DEG), np.float32)], axis=1
        )  # [E, NPAD, DEG]
        masknm = np.ascontiguousarray(
            mpad.reshape(E, NBLK, 128, DEG).transpose(2, 0, 1, 3).astype(np.float32)
        )  # [128, E, NBLK, DEG]

        # compacted gather: per (edge, block) keep only valid (n, d) pairs,
        # sorted by table row for HBM locality; selection matrix A recovers
        # the per-node sum (recip scale applied later at PSUM eviction).
        adt = TDH if FP8 else BF16
        idxc = np.full((E, NBLK, CIDX), ZROW, np.int32)
        amat = np.zeros((NBLK, 128, E, TPAD, 128), adt)
        selb = sel.reshape(E, NBLK, 128, DEG)
        mskb = mpad.reshape(E, NBLK, 128, DEG) > 0
        for e in range(E):
            for blk in range(NBLK):
                nn, dd = np.nonzero(mskb[e, blk])  # n-local, d of valid pairs
                vals = selb[e, blk, nn, dd]
                order = np.argsort(vals, kind="stable")
                V = len(order)
                assert V <= CIDX, f"TPAD too small: {V} > {CIDX}"
                idxc[e, blk, :V] = vals[order]
                a = np.zeros((CIDX, 128), adt)
                a[np.arange(V), nn[order]] = 1
                amat[blk, :, e, :, :] = a.reshape(TPAD, 128, 128).transpose(1, 0, 2)
        wv = idxc.reshape(E, NBLK, CIDX // 16, 16).transpose(0, 1, 3, 2)
        idxw = np.tile(wv, (1, 1, 8, 1)).astype(np.int16)  # [E, NBLK, 128, CIDX//16]
        percore.append((idxw, masknm, amat))
    return shared, percore


def _prep_core(h, tbl, shared, percore, core):
    n0 = core * NLOC
    hp = np.zeros((B, NPAD, F), np.float32)
    hp[:, :NLOC, :] = h[:, n0 : n0 + NLOC, :]
    # hT[f, col] with col = blk*512 + b*128 + n
    hTl = np.ascontiguousarray(
        hp.reshape(B, NBLK, 128, F).transpose(3, 1, 0, 2).reshape(F, COLS)
    )
    idxw, masknm, amat = percore[core]
    m = dict(tbl=tbl, hT=hTl, idxw=idxw, masknm=masknm, amat=amat)
    m.update(shared)
    return m


def _make_table(h):
    dt = TDH if FP8 else BF16
    tbl = np.zeros((N + 1, EW), dt)
    tbl[:N] = h.transpose(1, 0, 2).reshape(N, EW).astype(dt)
    return tbl


def kernel(**inputs):
    h = np.asarray(inputs["h"], np.float32)
    nc = _get_program()
    shared, percore = _prep_shared(
        np.asarray(inputs["edge_indices"]),
        np.asarray(inputs["edge_masks"], np.float32),
        np.asarray(inputs["W_edge"], np.float32),
        np.asarray(inputs["b_edge"], np.float32),
        np.asarray(inputs["ln_scale"], np.float32),
        np.asarray(inputs["ln_bias"], np.float32),
        np.asarray(inputs["W1"], np.float32),
        np.asarray(inputs["b1"], np.float32),
        np.asarray(inputs["W2"], np.float32),
        np.asarray(inputs["b2"], np.float32),
    )
    tbl = _make_table(h)
    in_maps = [_prep_core(h, tbl, shared, percore, core) for core in range(NCORES)]

    res = run_bass_kernel_spmd(nc, in_maps, core_ids=list(range(NCORES)))

    out = np.empty((B, N, F), np.float32)
    for core in range(NCORES):
        n0 = core * NLOC
        o = res.results[core]["outT"]  # [256, COLS]
        arr = o.reshape(F, NBLK, B, 128).transpose(2, 1, 3, 0).reshape(B, NPAD, F)
        out[:, n0 : n0 + NLOC, :] = arr[:, :NLOC, :]
    return out


# revision 11
# speedup vs baseline: 1.2700x; 1.0240x over previous
"""MultiEdgeGraphBlock kernel for 8 Trainium2 NeuronCores — v2.

Sharding: nodes across cores (1250/core, padded 1280), ALL batches per core.
The gather table fuses all 4 batches per node into one 2048-byte bf16 row
(row m = [h[0,m,:] h[1,m,:] h[2,m,:] h[3,m,:]]), so each SWDGE gather
descriptor moves 2048B instead of 512B and the per-core descriptor count
drops 4x vs the (batch, node-half) sharding.

Per core, per 128-node block (10 blocks):
  - for each edge type: one 2048-index dma_gather -> G[128, 16, 1024] bf16
    (node-major; j = d*128 + n index layout).
  - DEG-sum on PE: 16 identity matmuls x 2 psum halves (free=512).
  - mean via ACT eviction with per-partition reciprocal-degree scale.
  - PE transpose to feature-major (8 x 128x128), then aggregated^T
    accumulates W_i^T @ meanT in PSUM over edges.
  - LayerNorm feature-major: stats via ones-vector matmuls; mu/rstd
    broadcast across partitions with a K=1 ones-row matmul (no DRAM trip).
  - MLP as feature-major bf16 matmuls; residual add in f32.
Columns within a block: col = b*128 + n  (512 LN/MLP columns per block).
"""

import os
import sys

sys.path.insert(0, "/opt/trn_rl_repo")

import numpy as np
import ml_dtypes
from contextlib import ExitStack

import concourse.bass as bass
import concourse.mybir as mybir
import concourse.tile as tile
from concourse import bacc
from concourse.bass_utils import run_bass_kernel_spmd

BF16 = ml_dtypes.bfloat16
F32 = mybir.dt.float32
BF = mybir.dt.bfloat16
I16 = mybir.dt.int16
AO = mybir.AluOpType
AF = mybir.ActivationFunctionType

B, N, F, E, DEG, H = 4, 10000, 256, 5, 16, 256
NCORES = 8
NLOC = N // NCORES     # 1250 nodes per core
NPAD = 1280            # padded to 10 blocks of 128
NBLK = NPAD // 128     # 10
COLS = NPAD * B        # 5120 LN/MLP columns per core
BLK = 512              # columns per block (128 nodes x 4 batches)
ZROW = N               # zero-row index in gather table
EW = B * F             # 1024 elements per fused table row
GIDX = DEG * 128       # 2048 indices per gather call
TPAD = 9               # compact: gather tiles of 128 rows per (edge, block)
CIDX = TPAD * 128      # 1152 compacted indices per gather call
NQUEUES = 4
LN_EPS = 1e-6

_PROGRAM = {}
VARIANT = "full"  # full | gather | nogather
FP8 = True        # gather table in fp8 e4m3 (halves gather HBM traffic)
SCRATCH = int(os.environ.get("KSCRATCH", "32768"))
SINGLE_PACKET = os.environ.get("KSP", "0") == "1"
DR = os.environ.get("KDR", "1") == "1"  # DoubleRow fp8 reduce matmuls
NEGPAD = os.environ.get("KNEG", "0") == "1"  # pad gather idx with -1 (skipped)

TD = mybir.dt.float8e4
TDH = ml_dtypes.float8_e4m3


def _build_program(repeat=1):
    nc = bacc.Bacc(
        "TRN2",
        target_bir_lowering=False,
        debug=False,
        dynamic_dma_scratch_size=SCRATCH,
        num_swdge_queues=NQUEUES,
    )

    tbl = nc.dram_tensor("tbl", [N + 1, EW], TD if FP8 else BF, kind="ExternalInput")
    hT = nc.dram_tensor("hT", [F, COLS], F32, kind="ExternalInput")
    idxw = nc.dram_tensor("idxw", [E, NBLK, 128, CIDX // 16], I16, kind="ExternalInput")
    amat = nc.dram_tensor(
        "amat", [NBLK, 128, E, TPAD, 128], TD if FP8 else BF, kind="ExternalInput"
    )
    masknm = nc.dram_tensor("masknm", [128, E, NBLK, DEG], F32, kind="ExternalInput")
    w_pe = nc.dram_tensor("w_pe", [128, E * 4, 128], BF, kind="ExternalInput")
    w1 = nc.dram_tensor("w1", [128, 8, 128], BF, kind="ExternalInput")
    w2 = nc.dram_tensor("w2", [128, 4, 128], BF, kind="ExternalInput")
    ident_d = nc.dram_tensor("ident", [128, 128], BF, kind="ExternalInput")
    identq_d = nc.dram_tensor("identq", [128, 128], TD, kind="ExternalInput")
    ones1_d = nc.dram_tensor("ones1", [128, 1], BF, kind="ExternalInput")
    onesr_d = nc.dram_tensor("onesr", [1, 128], F32, kind="ExternalInput")
    b1_d = nc.dram_tensor("b1pc", [128, 2], F32, kind="ExternalInput")
    b2_d = nc.dram_tensor("b2pc", [128, 2], F32, kind="ExternalInput")
    lns_d = nc.dram_tensor("lnspc", [128, 4], F32, kind="ExternalInput")
    lnb_d = nc.dram_tensor("lnbpc", [128, 4], F32, kind="ExternalInput")
    bedg_d = nc.dram_tensor("bedgpc", [128, 2, E], F32, kind="ExternalInput")

    outT = nc.dram_tensor("outT", [F, COLS], F32, kind="ExternalOutput")

    with tile.TileContext(nc) as tc, ExitStack() as ctx:
        cpool = ctx.enter_context(tc.tile_pool(name="const", bufs=1))
        spsum = ctx.enter_context(tc.tile_pool(name="spsum", bufs=2, space="PSUM"))
        mtpsum = ctx.enter_context(tc.tile_pool(name="mtpsum", bufs=2, space="PSUM"))
        apsum = ctx.enter_context(tc.tile_pool(name="apsum", bufs=1, space="PSUM"))
        mlpsum = ctx.enter_context(tc.tile_pool(name="mlpsum", bufs=1, space="PSUM"))
        gpool = ctx.enter_context(tc.tile_pool(name="g", bufs=6))
        ipool = ctx.enter_context(tc.tile_pool(name="idx", bufs=8))
        xpool = ctx.enter_context(tc.tile_pool(name="x", bufs=2))
        apool = ctx.enter_context(tc.tile_pool(name="amat", bufs=2))
        wpool = ctx.enter_context(tc.tile_pool(name="work", bufs=2))

        # ---------------- constants ----------------
        W_sb = cpool.tile([128, E * 4, 128], BF)
        nc.sync.dma_start(W_sb[:], w_pe[:])
        W1_sb = cpool.tile([128, 8, 128], BF)
        nc.sync.dma_start(W1_sb[:], w1[:])
        W2_sb = cpool.tile([128, 4, 128], BF)
        nc.sync.dma_start(W2_sb[:], w2[:])
        id_sb = cpool.tile([128, 128], BF)
        nc.sync.dma_start(id_sb[:], ident_d[:])
        idq_sb = cpool.tile([128, 128], TD)
        nc.sync.dma_start(idq_sb[:], identq_d[:])
        on_sb = cpool.tile([128, 1], BF)
        nc.sync.dma_start(on_sb[:], ones1_d[:])
        onr_sb = cpool.tile([1, 128], F32)
        nc.sync.dma_start(onr_sb[:], onesr_d[:])
        b1_sb = cpool.tile([128, 2], F32)
        nc.sync.dma_start(b1_sb[:], b1_d[:])
        b2_sb = cpool.tile([128, 2], F32)
        nc.sync.dma_start(b2_sb[:], b2_d[:])
        lns_sb = cpool.tile([128, 4], F32)
        nc.sync.dma_start(lns_sb[:], lns_d[:])
        lnb_sb = cpool.tile([128, 4], F32)
        nc.sync.dma_start(lnb_sb[:], lnb_d[:])
        bedg_sb = cpool.tile([128, 2, E], F32)
        nc.sync.dma_start(bedg_sb[:], bedg_d[:])
        bsum_sb = cpool.tile([128, 2], F32)
        nc.vector.tensor_reduce(
            bsum_sb[:], bedg_sb[:], axis=mybir.AxisListType.X, op=AO.add
        )

        # ---------------- reciprocal degree (node-major) ----------------
        mask_sb = cpool.tile([128, E, NBLK, DEG], F32)
        nc.sync.dma_start(mask_sb[:], masknm[:])
        dn_sb = cpool.tile([128, E, NBLK], F32)
        for i in range(E):
            nc.vector.tensor_reduce(
                dn_sb[:, i, :], mask_sb[:, i], axis=mybir.AxisListType.X, op=AO.add
            )
        nc.vector.tensor_scalar_max(dn_sb[:], dn_sb[:], 1.0)
        recip_sb = cpool.tile([128, E, NBLK], F32)
        nc.vector.reciprocal(recip_sb[:], dn_sb[:])

        qc = 0  # SWDGE queue round-robin counter

        if NEGPAD:
            # pre-fill all G buffers once: rows beyond the runtime index count
            # keep stale-but-finite data (A-matrix rows are 0 there).
            for _ in range(6):
                g0 = gpool.tile([128, TPAD, EW], TD if FP8 else BF, tag="G")
                nc.vector.memset(g0[:], 0.0)

        # ---------------- main loop over node blocks ----------------
        for rep in range(repeat):
            for blk in range(NBLK):
                ns = bass.ts(blk, BLK)
                x = xpool.tile([128, 4, BLK], F32)
                nc.sync.dma_start(x[:, 0, :], hT[0:128, ns])
                nc.sync.dma_start(x[:, 1, :], hT[128:256, ns])

                A_sb = apool.tile([128, E, TPAD, 128], TD if FP8 else BF, tag="A")
                nc.sync.dma_start(A_sb[:], amat[blk])
                agg = apsum.tile([128, 2, BLK], F32, tag="agg")
                for i in range(E):
                    idx_t = ipool.tile([128, CIDX // 16], I16)
                    nc.sync.dma_start(idx_t[:], idxw[i, blk])
                    G = gpool.tile([128, TPAD, EW], TD if FP8 else BF, tag="G")
                    ga = G[:]
                    gap = bass.AP(
                        ga.tensor, ga.offset, [ga.ap[0], [EW, TPAD], [1, EW]]
                    )
                    if VARIANT == "nogather":
                        nc.vector.memset(G[:, 0, 0:8], 0.0)
                    else:
                        nc.gpsimd.dma_gather(
                            out_ap=gap,
                            in_ap=tbl.ap(),
                            idxs_ap=idx_t[:],
                            num_idxs=CIDX,
                            num_idxs_reg=CIDX,
                            elem_size=EW,
                            single_packet=SINGLE_PACKET,
                            queue_num=qc % NQUEUES,
                        )
                    qc += 1
                    if VARIANT == "gather":
                        continue
                    mT = mtpsum.tile([128, 8, 128], BF, tag="mT")
                    for h2 in range(2):
                        S = spsum.tile([128, BLK], F32, tag="S")
                        if DR and FP8:
                            for p in range(TPAD // 2):
                                nc.tensor.matmul(
                                    S[:],
                                    A_sb[:, i, 2 * p : 2 * p + 2, :],
                                    G[:, 2 * p : 2 * p + 2,
                                      h2 * 512 : (h2 + 1) * 512],
                                    start=(p == 0),
                                    stop=False,
                                    perf_mode=mybir.MatmulPerfMode.DoubleRow,
                                )
                            nc.tensor.matmul(
                                S[:],
                                A_sb[:, i, TPAD - 1, :],
                                G[:, TPAD - 1, h2 * 512 : (h2 + 1) * 512],
                                start=False,
                                stop=True,
                            )
                        else:
                            for t in range(TPAD):
                                nc.tensor.matmul(
                                    S[:],
                                    A_sb[:, i, t, :],
                                    G[:, t, h2 * 512 : (h2 + 1) * 512],
                                    start=(t == 0),
                                    stop=(t == TPAD - 1),
                                )
                        # mean (node-major): per-partition reciprocal scale
                        mean = wpool.tile([128, BLK], BF, tag="mean")
                        nc.scalar.activation(
                            mean[:], S[:], AF.Copy,
                            scale=recip_sb[:, i, blk : blk + 1],
                        )
                        # transpose reduced means to feature-major
                        for q in range(4):
                            nc.tensor.transpose(
                                mT[:, h2 * 4 + q, :],
                                mean[:, q * 128 : (q + 1) * 128],
                                id_sb[:],
                            )
                    mT_sb = wpool.tile([128, 8, 128], BF, tag="mTsb")
                    nc.scalar.copy(mT_sb[:], mT[:])
                    # aggregated^T += W_i^T @ meanT  (per batch, f-half)
                    for b4 in range(B):
                        for m in range(2):
                            for c in range(2):
                                nc.tensor.matmul(
                                    agg[:, m, b4 * 128 : (b4 + 1) * 128],
                                    W_sb[:, (i * 2 + c) * 2 + m, :],
                                    mT_sb[:, b4 * 2 + c, :],
                                    start=(i == 0 and b4 == 0 and c == 0),
                                    stop=(i == E - 1 and b4 == B - 1 and c == 1),
                                )
                if VARIANT == "gather":
                    continue
                # aggregated -> x bottom half (+ sum of edge biases)
                for m in range(2):
                    nc.scalar.activation(
                        x[:, 2 + m, :],
                        agg[:, m, :],
                        AF.Identity,
                        bias=bsum_sb[:, m : m + 1],
                        scale=1.0,
                    )

                # ---------------- layer norm (features on partitions) ------
                st = mlpsum.tile([128, 2, BLK], F32, tag="mlp")
                xbs = []
                for c in range(4):
                    xb = wpool.tile([128, BLK], BF, tag=f"xb{c}")
                    nc.vector.tensor_copy(xb[:], x[:, c, :])
                    xbs.append(xb)
                    nc.tensor.matmul(
                        st[0:1, 0, :], on_sb[:], xb[:],
                        start=(c == 0), stop=(c == 3),
                    )
                for c in range(4):
                    xsq = wpool.tile([128, BLK], BF, tag="xsq")
                    nc.scalar.square(xsq[:], xbs[c][:])
                    nc.tensor.matmul(
                        st[0:1, 1, :], on_sb[:], xsq[:],
                        start=(c == 0), stop=(c == 3),
                    )
                mrow = wpool.tile([1, 2, BLK], F32, tag="mrow")
                nc.vector.tensor_scalar_mul(mrow[0:1, 0, :], st[0:1, 0, :], 1.0 / 512.0)
                mu2 = wpool.tile([1, BLK], F32, tag="mu2")
                nc.vector.tensor_mul(mu2[0:1, :], mrow[0:1, 0, :], mrow[0:1, 0, :])
                nc.vector.tensor_scalar_sub(mu2[0:1, :], mu2[0:1, :], LN_EPS)
                var = wpool.tile([1, BLK], F32, tag="var")
                nc.vector.scalar_tensor_tensor(
                    var[0:1, :], st[0:1, 1, :], 1.0 / 512.0, mu2[0:1, :],
                    op0=AO.mult, op1=AO.subtract,
                )
                sd = wpool.tile([1, BLK], F32, tag="sd")
                nc.scalar.activation(sd[0:1, :], var[0:1, :], AF.Sqrt, bias=0.0)
                nc.vector.reciprocal(mrow[0:1, 1, :], sd[0:1, :])
                # broadcast mu/rstd to all 128 partitions via K=1 matmul
                bc = mlpsum.tile([128, 2, BLK], F32, tag="mlp")
                for r in range(2):
                    nc.tensor.matmul(
                        bc[:, r, :], onr_sb[:], mrow[0:1, r, :],
                        start=True, stop=True,
                    )

                xln = wpool.tile([128, 4, BLK], BF, tag="xln")
                for c in range(4):
                    tt = wpool.tile([128, BLK], F32, tag="tt")
                    nc.vector.scalar_tensor_tensor(
                        tt[:], x[:, c, :], 0.0, bc[:, 0, :],
                        op0=AO.add, op1=AO.subtract,
                    )
                    nc.vector.tensor_mul(tt[:], tt[:], bc[:, 1, :])
                    nc.scalar.activation(
                        xln[:, c, :], tt[:], AF.Identity,
                        bias=lnb_sb[:, c : c + 1], scale=lns_sb[:, c : c + 1],
                    )

                # ---------------- MLP ----------------
                y1 = mlpsum.tile([128, 2, BLK], F32, tag="mlp")
                for m in range(2):
                    for k in range(4):
                        nc.tensor.matmul(
                            y1[:, m, :], W1_sb[:, k * 2 + m, :], xln[:, k, :],
                            start=(k == 0), stop=(k == 3),
                        )
                y1b = wpool.tile([128, 2, BLK], BF, tag="y1b")
                for m in range(2):
                    nc.scalar.activation(
                        y1b[:, m, :], y1[:, m, :], AF.Relu,
                        bias=b1_sb[:, m : m + 1], scale=1.0,
                    )
                y2 = mlpsum.tile([128, 2, BLK], F32, tag="mlp")
                for m in range(2):
                    for k in range(2):
                        nc.tensor.matmul(
                            y2[:, m, :], W2_sb[:, k * 2 + m, :], y1b[:, k, :],
                            start=(k == 0), stop=(k == 1),
                        )
                ot = wpool.tile([128, 2, BLK], F32, tag="ot")
                for m in range(2):
                    nc.vector.scalar_tensor_tensor(
                        ot[:, m, :], y2[:, m, :], b2_sb[:, m : m + 1], x[:, m, :],
                        op0=AO.add, op1=AO.add,
                    )
                for m in range(2):
                    nc.sync.dma_start(outT[m * 128 : (m + 1) * 128, ns], ot[:, m, :])

    nc.compile()
    return nc


def _get_program(repeat=1):
    key = (repeat, VARIANT)
    if key not in _PROGRAM:
        _PROGRAM[key] = _build_program(repeat)
    return _PROGRAM[key]


def _prep_shared(edge_indices, edge_masks, W_edge, b_edge, ln_scale, ln_bias,
                 W1, b1, W2, b2):
    """Host-side layout prep: shared weights + per-core index/mask slices."""
    W_pe = np.empty((128, E * 4, 128), np.float32)
    for i in range(E):
        for c in range(2):
            for m in range(2):
                W_pe[:, (i * 2 + c) * 2 + m, :] = W_edge[
                    i, c * 128 : (c + 1) * 128, m * 128 : (m + 1) * 128
                ]
    W1b = np.empty((128, 8, 128), np.float32)
    for k in range(4):
        for m in range(2):
            W1b[:, k * 2 + m, :] = W1[k * 128 : (k + 1) * 128, m * 128 : (m + 1) * 128]
    W2b = np.empty((128, 4, 128), np.float32)
    for k in range(2):
        for m in range(2):
            W2b[:, k * 2 + m, :] = W2[k * 128 : (k + 1) * 128, m * 128 : (m + 1) * 128]

    shared = dict(
        w_pe=W_pe.astype(BF16),
        w1=W1b.astype(BF16),
        w2=W2b.astype(BF16),
        ident=np.eye(128, dtype=BF16),
        identq=np.eye(128, dtype=TDH),
        ones1=np.ones((128, 1), BF16),
        onesr=np.ones((1, 128), np.float32),
        b1pc=np.ascontiguousarray(b1.reshape(2, 128).T.astype(np.float32)),
        b2pc=np.ascontiguousarray(b2.reshape(2, 128).T.astype(np.float32)),
        lnspc=np.ascontiguousarray(ln_scale.reshape(4, 128).T.astype(np.float32)),
        lnbpc=np.ascontiguousarray(ln_bias.reshape(4, 128).T.astype(np.float32)),
        bedgpc=np.ascontiguousarray(
            b_edge.T.reshape(2, 128, E).transpose(1, 0, 2).astype(np.float32)
        ),
    )

    # per core: gather indices (mask-select -> zero row), node-major masks
    percore = []
    for core in range(NCORES):
        n0 = core * NLOC
        idx = edge_indices[:, n0 : n0 + NLOC, :].astype(np.int64)  # [E, NLOC, DEG]
        msk = edge_masks[:, n0 : n0 + NLOC, :]
        idx = np.where(idx < 0, 0, idx)
        sel = np.where(msk > 0, idx, ZROW).astype(np.int32)
        sel = np.concatenate(
            [sel, np.full((E, NPAD - NLOC, DEG), ZROW, np.int32)], axis=1
        )  # [E, NPAD, DEG]
        mpad = np.concatenate(
            [msk, np.zeros((E, NPAD - NLOC, DEG), np.float32)], axis=1
        )  # [E, NPAD, DEG]
        masknm = np.ascontiguousarray(
            mpad.reshape(E, NBLK, 128, DEG).transpose(2, 0, 1, 3).astype(np.float32)
        )  # [128, E, NBLK, DEG]

        # compacted gather: per (edge, block) keep only valid (n, d) pairs,
        # sorted by table row for HBM locality; selection matrix A recovers
        # the per-node sum (recip scale applied later at PSUM eviction).
        adt = TDH if FP8 else BF16
        idxc = np.full((E, NBLK, CIDX), -1 if NEGPAD else ZROW, np.int32)
        amat = np.zeros((NBLK, 128, E, TPAD, 128), adt)
        selb = sel.reshape(E, NBLK, 128, DEG)
        mskb = mpad.reshape(E, NBLK, 128, DEG) > 0
        for e in range(E):
            for blk in range(NBLK):
                nn, dd = np.nonzero(mskb[e, blk])  # n-local, d of valid pairs
                vals = selb[e, blk, nn, dd]
                uniq, inv = np.unique(vals, return_inverse=True)
                V = len(uniq)
                assert V <= CIDX, f"TPAD too small: {V} > {CIDX}"
                idxc[e, blk, :V] = uniq
                a = np.zeros((CIDX, 128), np.float32)
                np.add.at(a, (inv, nn), 1.0)
                amat[blk, :, e, :, :] = (
                    a.astype(adt).reshape(TPAD, 128, 128).transpose(1, 0, 2)
                )
        wv = idxc.reshape(E, NBLK, CIDX // 16, 16).transpose(0, 1, 3, 2)
        idxw = np.tile(wv, (1, 1, 8, 1)).astype(np.int16)  # [E, NBLK, 128, CIDX//16]
        percore.append((idxw, masknm, amat))
    return shared, percore


def _prep_core(h, tbl, shared, percore, core):
    n0 = core * NLOC
    hp = np.zeros((B, NPAD, F), np.float32)
    hp[:, :NLOC, :] = h[:, n0 : n0 + NLOC, :]
    # hT[f, col] with col = blk*512 + b*128 + n
    hTl = np.ascontiguousarray(
        hp.reshape(B, NBLK, 128, F).transpose(3, 1, 0, 2).reshape(F, COLS)
    )
    idxw, masknm, amat = percore[core]
    m = dict(tbl=tbl, hT=hTl, idxw=idxw, masknm=masknm, amat=amat)
    m.update(shared)
    return m


def _make_table(h):
    dt = TDH if FP8 else BF16
    tbl = np.zeros((N + 1, EW), dt)
    tbl[:N] = h.transpose(1, 0, 2).reshape(N, EW).astype(dt)
    return tbl


def kernel(**inputs):
    h = np.asarray(inputs["h"], np.float32)
    nc = _get_program()
    shared, percore = _prep_shared(
        np.asarray(inputs["edge_indices"]),
        np.asarray(inputs["edge_masks"], np.float32),
        np.asarray(inputs["W_edge"], np.float32),
        np.asarray(inputs["b_edge"], np.float32),
        np.asarray(inputs["ln_scale"], np.float32),
        np.asarray(inputs["ln_bias"], np.float32),
        np.asarray(inputs["W1"], np.float32),
        np.asarray(inputs["b1"], np.float32),
        np.asarray(inputs["W2"], np.float32),
        np.asarray(inputs["b2"], np.float32),
    )
    tbl = _make_table(h)
    in_maps = [_prep_core(h, tbl, shared, percore, core) for core in range(NCORES)]

    res = run_bass_kernel_spmd(nc, in_maps, core_ids=list(range(NCORES)))

    out = np.empty((B, N, F), np.float32)
    for core in range(NCORES):
        n0 = core * NLOC
        o = res.results[core]["outT"]  # [256, COLS]
        arr = o.reshape(F, NBLK, B, 128).transpose(2, 1, 3, 0).reshape(B, NPAD, F)
        out[:, n0 : n0 + NLOC, :] = arr[:, :NLOC, :]
    return out
